# revision 31
# baseline (speedup 1.0000x reference)
"""Trainium2 Bass kernel for nn_AMM_module_55027120996423.

Computation: 3->1 channel 3x3 'same' conv + bias; softmax over the single
channel == 1.0; output hi = where(conv(x,w) + b < -0.5, 0, 1) as float32.

Strategy: pure data parallel over batch (32 images -> 4 per core x 8
cores), no collectives.  Per core the conv runs on the TensorEngine as
banded-Toeplitz matmuls: each 126-row output tile takes 9 accumulating
matmuls (3 channels x 3 horizontal taps kx).  The stationary operand is a
[128, 126] fp16 band matrix carrying the 3 vertical taps (ky) on its
diagonals; horizontal taps come free as column-offset slices of the
moving operand (one zero halo column per side, zero pad row for the top
image tile).  The 8-row tails of all 4 images share one 108-partition
tile-set.  Matmuls run in fp16 (full PE rate + fast weight load); the f32->fp16
input conversion happens inline in gpsimd casting DMAs for half the
tiles and on the Scalar engine (after sync-queue f32 DMAs) for the other
half, so the two DMA-issue paths and the cast work are spread across
otherwise-idle engines.  fp16 rounding (~2^-12/term) keeps
threshold-flip noise ~1e-2 relative L2.
The threshold (is_ge against -(0.5+b)) runs on the Vector engine straight
out of PSUM and emits fp16 0.0/1.0 (exact), halving the output stream;
the host upcasts to float32.  Bands/bias are built on-chip from raw w/b.
"""

import os
from contextlib import ExitStack

import numpy as np

import concourse.tile as tile
from concourse import bacc, mybir
from concourse.bass_utils import run_bass_kernel_spmd

F32 = mybir.dt.float32
F16 = mybir.dt.float16

B, C, H, W = 32, 3, 512, 512
NCORES = 8
BPC = B // NCORES  # images per core

TILE_ROWS = 126            # output rows per main tile-set
NTILES = 4                 # main tile-sets per image (4*126 = 504 rows)
TAIL = H - NTILES * TILE_ROWS  # 8 rows in the shared tail set
CW = W + 2                 # channel block width (one halo column per side)

LAST_EXEC_NS = None
LAST_RESULTS = None

_cache = {}


def _build_nc():
    nc = bacc.Bacc("TRN2", target_bir_lowering=False, debug=False,
                   num_devices=NCORES)
    xp = nc.dram_tensor("x", [BPC, C, H, W], F32, kind="ExternalInput").ap()
    wp = nc.dram_tensor("w", [1, C, 3, 3], F32, kind="ExternalInput").ap()
    bp = nc.dram_tensor("b", [1], F32, kind="ExternalInput").ap()
    yp = nc.dram_tensor("out", [BPC, H, W], F16, kind="ExternalOutput").ap()

    # main tile-set order: interior tiles of image 0 first so the first
    # matmuls only wait on the band, then everything else
    SETS = [(0, 1), (0, 2), (0, 3), (0, 0)] + [
        (img, t) for img in range(1, BPC) for t in range(NTILES)]
    PRELOAD = 3  # tile-sets loaded ahead of compute in program order

    with tile.TileContext(nc) as tc, ExitStack() as ctx:
        const_pool = ctx.enter_context(tc.tile_pool(name="const", bufs=1))
        xf16_pool = ctx.enter_context(tc.tile_pool(name="xf16", bufs=PRELOAD + 2))
        xf32_pool = ctx.enter_context(tc.tile_pool(name="xf32", bufs=5))
        out_pool = ctx.enter_context(tc.tile_pool(name="outp", bufs=4))
        psum_pool = ctx.enter_context(tc.tile_pool(name="ps", bufs=5, space="PSUM"))
        psw_pool = ctx.enter_context(tc.tile_pool(name="psw", bufs=1, space="PSUM"))

        x_tiles = {}
        band_int = None
        band_top = None
        thr = None

        nload = [0]

        def emit_load(img, t):
            xt = xf16_pool.tile([128, C * CW], F16, tag="xt")
            xv = xt.rearrange("p (c q) -> p c q", c=C)
            # zero halo columns (image left/right 'same' padding)
            nc.vector.memset(xv[:, :, 0:1], 0.0)
            nc.vector.memset(xv[:, :, CW - 1:CW], 0.0)
            # tile 0 holds input rows 0..127 and uses band_top (whose
            # shifted diagonals supply the implicit row -1 zero padding);
            # tiles 1..3 hold rows 126t-1 .. 126t+126.
            r0 = 0 if t == 0 else TILE_ROWS * t - 1
            src = xp[img, :, r0:r0 + 128, :].rearrange("c r w -> r c w")
            if nload[0] % 2 == 0:
                # SWDGE path: casting DMA straight to fp16
                nc.gpsimd.dma_start(xv[:, :, 1:1 + W], src)
            else:
                # HWDGE path: f32 DMA on sync, fp16 cast on the Scalar eng
                xf = xf32_pool.tile([128, C * W], F32, tag="xf")
                xfv = xf.rearrange("p (c q) -> p c q", c=C)
                nc.sync.dma_start(xfv[:], src)
                nc.scalar.copy(xv[:, :, 1:1 + W], xfv[:])
            nload[0] += 1
            x_tiles[(img, t)] = xt

        def emit_compute(img, t):
            xt = x_tiles.pop((img, t))
            pt = psum_pool.tile([TILE_ROWS, W], F32, tag="pt")
            band = band_top if t == 0 else band_int
            for c in range(C):
                for kx in range(3):
                    blk = c * 3 + kx
                    nc.tensor.matmul(
                        pt[:],
                        band[:, blk * TILE_ROWS:(blk + 1) * TILE_ROWS],
                        xt[:, c * CW + kx:c * CW + kx + W],
                        start=(blk == 0), stop=(blk == 8),
                    )
            ot = out_pool.tile([TILE_ROWS, W], F16, tag="ot")
            nc.vector.tensor_scalar(out=ot[:], in0=pt[:],
                                    scalar1=thr[0:TILE_ROWS, 0:1],
                                    scalar2=None,
                                    op0=mybir.AluOpType.is_ge)
            nc.scalar.dma_start(
                yp[img, t * TILE_ROWS:(t + 1) * TILE_ROWS, :], ot[:])

        # ---- diagonal masks (no data dependence, built before w arrives) --
        # M_d[p, m] = 1.0 where p - m == d, d in {-1, 0, 1, 2}
        diag = const_pool.tile([128, TILE_ROWS], mybir.dt.int16)
        nc.gpsimd.iota(diag[:], pattern=[[-1, TILE_ROWS]], base=0,
                       channel_multiplier=1)
        masks = {}
        for d in (-1, 0, 1, 2):
            m = const_pool.tile([128, TILE_ROWS], F16, tag=f"mask{d}")
            nc.vector.tensor_scalar(out=m[:], in0=diag[:], scalar1=d,
                                    scalar2=None,
                                    op0=mybir.AluOpType.is_equal)
            masks[d] = m

        # ---- weight prep (once) -------------------------------------------
        # wb col (ky*9 + c*3 + kx) = w[0,c,ky,kx]; col 27 = b; replicated to
        # all 128 partitions directly by the DMA
        w_sb = const_pool.tile([1, 28], F32)
        nc.sync.dma_start(
            w_sb[0:1, 0:27].rearrange("a (ky c kx) -> a ky c kx",
                                      ky=3, c=3, kx=3),
            wp.rearrange("a c ky kx -> a ky c kx"))
        nc.sync.dma_start(w_sb[0:1, 27:28], bp.unsqueeze(0))
        ones_sb = const_pool.tile([1, 128], F32)
        nc.vector.memset(ones_sb[:], 1.0)
        psw = psw_pool.tile([128, 28], F32)
        nc.tensor.matmul(psw[:], ones_sb[:], w_sb[:], start=True, stop=True)
        wb = const_pool.tile([128, 28], F32)
        nc.vector.tensor_copy(wb[:], psw[:])
        wb16 = const_pool.tile([128, 28], F16)
        nc.vector.tensor_copy(wb16[:], psw[:])

        # threshold = -(0.5 + b), one copy per partition
        thr = const_pool.tile([128, 1], F32)
        nc.vector.tensor_scalar(out=thr[:], in0=wb[:, 27:28],
                                scalar1=-1.0, scalar2=-0.5,
                                op0=mybir.AluOpType.mult,
                                op1=mybir.AluOpType.add)

        # ---- first loads go ahead of the band build -----------------------
        for img, t in SETS[:PRELOAD]:
            emit_load(img, t)

        # ---- band construction (once) -------------------------------------
        # band[k, blk*126+m] = w[0,c,k-m,kx] for k-m in {0,1,2}, blk=c*3+kx.
        # Per ky, affine_select keeps the broadcast fp16 weight on the
        # k-m == ky diagonal; the three fields have disjoint support so the
        # two adds are exact.
        BW = 9 * TILE_ROWS
        band_int = const_pool.tile([128, BW], F16)
        band_top = const_pool.tile([128, BW], F16)
        # band fields: per (ky, blk) tensor_scalar mask * w (scalar AP), then
        # two exact adds (disjoint diagonal supports)
        fi = []
        for ky in range(3):
            f = const_pool.tile([128, BW], F16, tag=f"bandf{ky}")
            for blk in range(9):
                nc.vector.tensor_scalar(
                    out=f[:, blk * TILE_ROWS:(blk + 1) * TILE_ROWS],
                    in0=masks[ky][:], scalar1=wb[:, ky * 9 + blk:ky * 9 + blk + 1],
                    scalar2=None, op0=mybir.AluOpType.mult)
            fi.append(f)
        nc.vector.tensor_add(fi[0][:], fi[0][:], fi[1][:])
        nc.vector.tensor_add(band_int[:], fi[0][:], fi[2][:])
        # band_top (shift=1 variant) via gpsimd selects, in parallel
        ft = []
        for ky in range(3):
            f = const_pool.tile([128, BW], F16, tag=f"bandt{ky}")
            for blk in range(9):
                nc.vector.tensor_scalar(
                    out=f[:, blk * TILE_ROWS:(blk + 1) * TILE_ROWS],
                    in0=masks[ky - 1][:], scalar1=wb[:, ky * 9 + blk:ky * 9 + blk + 1],
                    scalar2=None, op0=mybir.AluOpType.mult)
            ft.append(f)
        nc.vector.tensor_add(ft[0][:], ft[0][:], ft[1][:])
        nc.vector.tensor_add(band_top[:], ft[0][:], ft[2][:])

        # tail band: partitions p = i*27 + c*9 + rr (input row 503+rr),
        # columns kx*32 + i*8 + j (out row 504+j).  Each (i,c) block is the
        # [9, 8] top-left corner of the interior band for that (c,kx).
        band_tail = const_pool.tile([128, 3 * BPC * TAIL], F16)
        nc.vector.memset(band_tail[:], 0.0)
        btv = band_tail.rearrange("p (kx i j) -> p kx i j", kx=3, i=BPC)
        biv = band_int.rearrange("p (b m) -> p b m", b=9)
        for i in range(BPC):
            for c in range(C):
                p0 = i * 27 + c * 9
                src = biv[0:9, c * 3:c * 3 + 3, 0:TAIL]       # [9, 3kx, 8]
                nc.sync.dma_start(btv[p0:p0 + 9, :, i, :], src)

        # ---- shared tail tile (rows 504..511 of all 4 images) -------------
        xtail = const_pool.tile([108, CW], F16)
        nc.vector.memset(xtail[:, 0:1], 0.0)
        nc.vector.memset(xtail[:, CW - 1:CW], 0.0)
        r0 = NTILES * TILE_ROWS - 1  # 503
        for i in range(BPC):
            for c in range(C):
                p0 = i * 27 + c * 9
                nc.gpsimd.dma_start(xtail[p0:p0 + 9, 1:1 + W],
                                    xp[i, c, r0:r0 + 9, :])

        MT = BPC * TAIL  # 32 output rows
        ptail = psw_pool.tile([MT, W], F32, tag="ptail")
        for kx in range(3):
            nc.tensor.matmul(
                ptail[:],
                band_tail[0:108, kx * MT:(kx + 1) * MT],
                xtail[0:108, kx:kx + W],
                start=(kx == 0), stop=(kx == 2),
            )
        otail = out_pool.tile([MT, W], F16, tag="otail")
        nc.vector.tensor_scalar(out=otail[:], in0=ptail[:],
                                scalar1=thr[0:MT, 0:1], scalar2=None,
                                op0=mybir.AluOpType.is_ge)
        for i in range(BPC):
            nc.scalar.dma_start(yp[i, NTILES * TILE_ROWS:H, :],
                                otail[i * TAIL:(i + 1) * TAIL, :])

        # ---- main tiles, software-pipelined program order -----------------
        for i, (img, t) in enumerate(SETS):
            nxt = i + PRELOAD
            if nxt < len(SETS):
                emit_load(*SETS[nxt])
            emit_compute(img, t)

    nc.compile()
    return nc


def kernel(x: np.ndarray, w: np.ndarray, b: np.ndarray) -> np.ndarray:
    global LAST_EXEC_NS, LAST_RESULTS
    if "nc" not in _cache:
        _cache["nc"] = _build_nc()
    nc = _cache["nc"]

    x = np.ascontiguousarray(x, dtype=np.float32)
    w = np.ascontiguousarray(w, dtype=np.float32)
    b = np.ascontiguousarray(b, dtype=np.float32)
    in_maps = [
        {"x": x[i * BPC:(i + 1) * BPC], "w": w, "b": b} for i in range(NCORES)
    ]

    kwargs = {}
    if os.environ.get("BASS_CONV_TRACE", "") not in ("", "0"):
        try:
            import ntff_shim
            ntff_shim.install()
            kwargs["trace"] = True
        except Exception:
            pass

    res = None
    for attempt in range(3):
        try:
            res = run_bass_kernel_spmd(nc, in_maps,
                                       core_ids=list(range(NCORES)), **kwargs)
            break
        except Exception:
            if attempt == 2:
                raise
    LAST_EXEC_NS = res.exec_time_ns
    LAST_RESULTS = res
    out = np.concatenate([res.results[i]["out"][:, None, :, :]
                          for i in range(NCORES)], axis=0)
    return out.astype(np.float32)


# revision 32
# speedup vs baseline: 1.0244x; 1.0244x over previous
"""Trainium2 Bass kernel for nn_AMM_module_55027120996423.

Computation: 3->1 channel 3x3 'same' conv + bias; softmax over the single
channel == 1.0; output hi = where(conv(x,w) + b < -0.5, 0, 1) as float32.

Strategy: pure data parallel over batch (32 images -> 4 per core x 8
cores), no collectives.  Per core the conv runs on the TensorEngine as
banded-Toeplitz matmuls: each 126-row output tile takes 9 accumulating
matmuls (3 channels x 3 horizontal taps kx).  The stationary operand is a
[128, 126] fp16 band matrix carrying the 3 vertical taps (ky) on its
diagonals; horizontal taps come free as column-offset slices of the
moving operand (one zero halo column per side, zero pad row for the top
image tile).  The 8-row tails of all 4 images share one 108-partition
tile-set.  Matmuls run in fp16 (full PE rate + fast weight load); the f32->fp16
input conversion happens inline in gpsimd casting DMAs for half the
tiles and on the Scalar engine (after sync-queue f32 DMAs) for the other
half, so the two DMA-issue paths and the cast work are spread across
otherwise-idle engines.  fp16 rounding (~2^-12/term) keeps
threshold-flip noise ~1e-2 relative L2.
The threshold (is_ge against -(0.5+b)) runs on the Vector engine straight
out of PSUM and emits fp16 0.0/1.0 (exact), halving the output stream;
the host upcasts to float32.  Bands/bias are built on-chip from raw w/b.
"""

import os
from contextlib import ExitStack

import numpy as np

import concourse.tile as tile
from concourse import bacc, mybir
from concourse.bass_utils import run_bass_kernel_spmd

F32 = mybir.dt.float32
F16 = mybir.dt.float16

B, C, H, W = 32, 3, 512, 512
NCORES = 8
BPC = B // NCORES  # images per core

TILE_ROWS = 126            # output rows per main tile-set
NTILES = 4                 # main tile-sets per image (4*126 = 504 rows)
TAIL = H - NTILES * TILE_ROWS  # 8 rows in the shared tail set
CW = W + 2                 # channel block width (one halo column per side)

LAST_EXEC_NS = None
LAST_RESULTS = None

_cache = {}


def _build_nc():
    nc = bacc.Bacc("TRN2", target_bir_lowering=False, debug=False,
                   num_devices=NCORES)
    xp = nc.dram_tensor("x", [BPC, C, H, W], F32, kind="ExternalInput").ap()
    wp = nc.dram_tensor("w", [1, C, 3, 3], F32, kind="ExternalInput").ap()
    bp = nc.dram_tensor("b", [1], F32, kind="ExternalInput").ap()
    yp = nc.dram_tensor("out", [BPC, H, W], F16, kind="ExternalOutput").ap()

    # main tile-set order: interior tiles of image 0 first so the first
    # matmuls only wait on the band, then everything else
    SETS = [(0, 1), (0, 2), (0, 3), (0, 0)] + [
        (img, t) for img in range(1, BPC) for t in range(NTILES)]
    PRELOAD = 3  # tile-sets loaded ahead of compute in program order

    with tile.TileContext(nc) as tc, ExitStack() as ctx:
        const_pool = ctx.enter_context(tc.tile_pool(name="const", bufs=1))
        xf16_pool = ctx.enter_context(tc.tile_pool(name="xf16", bufs=PRELOAD + 2))
        xf32_pool = ctx.enter_context(tc.tile_pool(name="xf32", bufs=4))
        out_pool = ctx.enter_context(tc.tile_pool(name="outp", bufs=4))
        psum_pool = ctx.enter_context(tc.tile_pool(name="ps", bufs=5, space="PSUM"))
        psw_pool = ctx.enter_context(tc.tile_pool(name="psw", bufs=1, space="PSUM"))

        x_tiles = {}
        band_int = None
        band_top = None
        thr = None

        nload = [0]

        def emit_load(img, t):
            xt = xf16_pool.tile([128, C * CW], F16, tag="xt")
            xv = xt.rearrange("p (c q) -> p c q", c=C)
            # zero halo columns (image left/right 'same' padding)
            nc.vector.memset(xv[:, :, 0:1], 0.0)
            nc.vector.memset(xv[:, :, CW - 1:CW], 0.0)
            # tile 0 holds input rows 0..127 and uses band_top (whose
            # shifted diagonals supply the implicit row -1 zero padding);
            # tiles 1..3 hold rows 126t-1 .. 126t+126.
            r0 = 0 if t == 0 else TILE_ROWS * t - 1
            src = xp[img, :, r0:r0 + 128, :].rearrange("c r w -> r c w")
            if nload[0] % 2 == 0:
                # SWDGE path: casting DMA straight to fp16
                nc.gpsimd.dma_start(xv[:, :, 1:1 + W], src)
            else:
                # HWDGE path: f32 DMA on sync, fp16 cast on the Scalar eng
                xf = xf32_pool.tile([128, C * W], F32, tag="xf")
                xfv = xf.rearrange("p (c q) -> p c q", c=C)
                nc.sync.dma_start(xfv[:], src)
                nc.scalar.copy(xv[:, :, 1:1 + W], xfv[:])
            nload[0] += 1
            x_tiles[(img, t)] = xt

        def emit_compute(img, t):
            xt = x_tiles.pop((img, t))
            pt = psum_pool.tile([TILE_ROWS, W], F32, tag="pt")
            band = band_top if t == 0 else band_int
            for c in range(C):
                for kx in range(3):
                    blk = c * 3 + kx
                    nc.tensor.matmul(
                        pt[:],
                        band[:, blk * TILE_ROWS:(blk + 1) * TILE_ROWS],
                        xt[:, c * CW + kx:c * CW + kx + W],
                        start=(blk == 0), stop=(blk == 8),
                    )
            ot = out_pool.tile([TILE_ROWS, W], F16, tag="ot")
            nc.vector.tensor_scalar(out=ot[:], in0=pt[:],
                                    scalar1=thr[0:TILE_ROWS, 0:1],
                                    scalar2=None,
                                    op0=mybir.AluOpType.is_ge)
            nc.scalar.dma_start(
                yp[img, t * TILE_ROWS:(t + 1) * TILE_ROWS, :], ot[:])

        # ---- diagonal masks (no data dependence, built before w arrives) --
        # M_d[p, m] = 1.0 where p - m == d, d in {-1, 0, 1, 2}
        diag = const_pool.tile([128, TILE_ROWS], mybir.dt.int16)
        nc.gpsimd.iota(diag[:], pattern=[[-1, TILE_ROWS]], base=0,
                       channel_multiplier=1)
        masks = {}
        for d in (-1, 0, 1, 2):
            m = const_pool.tile([128, TILE_ROWS], F16, tag=f"mask{d}")
            nc.vector.tensor_scalar(out=m[:], in0=diag[:], scalar1=d,
                                    scalar2=None,
                                    op0=mybir.AluOpType.is_equal)
            masks[d] = m

        # ---- weight prep (once) -------------------------------------------
        # wb col (ky*9 + c*3 + kx) = w[0,c,ky,kx]; col 27 = b; replicated to
        # all 128 partitions directly by the DMA
        w_sb = const_pool.tile([1, 28], F32)
        nc.sync.dma_start(
            w_sb[0:1, 0:27].rearrange("a (ky c kx) -> a ky c kx",
                                      ky=3, c=3, kx=3),
            wp.rearrange("a c ky kx -> a ky c kx"))
        nc.sync.dma_start(w_sb[0:1, 27:28], bp.unsqueeze(0))
        ones_sb = const_pool.tile([1, 128], F32)
        nc.vector.memset(ones_sb[:], 1.0)
        psw = psw_pool.tile([128, 28], F32)
        nc.tensor.matmul(psw[:], ones_sb[:], w_sb[:], start=True, stop=True)
        wb = const_pool.tile([128, 28], F32)
        nc.vector.tensor_copy(wb[:], psw[:])
        wb16 = const_pool.tile([128, 28], F16)
        nc.vector.tensor_copy(wb16[:], psw[:])

        # threshold = -(0.5 + b), one copy per partition
        thr = const_pool.tile([128, 1], F32)
        nc.vector.tensor_scalar(out=thr[:], in0=wb[:, 27:28],
                                scalar1=-1.0, scalar2=-0.5,
                                op0=mybir.AluOpType.mult,
                                op1=mybir.AluOpType.add)

        # ---- first loads go ahead of the band build -----------------------
        for img, t in SETS[:PRELOAD]:
            emit_load(img, t)

        # ---- band construction (once) -------------------------------------
        # band[k, blk*126+m] = w[0,c,k-m,kx] for k-m in {0,1,2}, blk=c*3+kx.
        # Per ky, affine_select keeps the broadcast fp16 weight on the
        # k-m == ky diagonal; the three fields have disjoint support so the
        # two adds are exact.
        BW = 9 * TILE_ROWS
        band_int = const_pool.tile([128, BW], F16)
        band_top = const_pool.tile([128, BW], F16)
        # band fields: per (ky, blk) tensor_scalar mask * w (scalar AP), then
        # two exact adds (disjoint diagonal supports)
        fi = []
        for ky in range(3):
            f = const_pool.tile([128, BW], F16, tag=f"bandf{ky}")
            for blk in range(9):
                nc.vector.tensor_scalar(
                    out=f[:, blk * TILE_ROWS:(blk + 1) * TILE_ROWS],
                    in0=masks[ky][:], scalar1=wb[:, ky * 9 + blk:ky * 9 + blk + 1],
                    scalar2=None, op0=mybir.AluOpType.mult)
            fi.append(f)
        nc.vector.tensor_add(fi[0][:], fi[0][:], fi[1][:])
        nc.vector.tensor_add(band_int[:], fi[0][:], fi[2][:])
        # band_top (shift=1 variant) via gpsimd selects, in parallel
        ft = []
        for ky in range(3):
            f = const_pool.tile([128, BW], F16, tag=f"bandt{ky}")
            for blk in range(9):
                nc.vector.tensor_scalar(
                    out=f[:, blk * TILE_ROWS:(blk + 1) * TILE_ROWS],
                    in0=masks[ky - 1][:], scalar1=wb[:, ky * 9 + blk:ky * 9 + blk + 1],
                    scalar2=None, op0=mybir.AluOpType.mult)
            ft.append(f)
        nc.vector.tensor_add(ft[0][:], ft[0][:], ft[1][:])
        nc.vector.tensor_add(band_top[:], ft[0][:], ft[2][:])

        # tail band: partitions p = i*27 + c*9 + rr (input row 503+rr),
        # columns kx*32 + i*8 + j (out row 504+j).  Each (i,c) block is the
        # [9, 8] top-left corner of the interior band for that (c,kx).
        band_tail = const_pool.tile([128, 3 * BPC * TAIL], F16)
        nc.vector.memset(band_tail[:], 0.0)
        btv = band_tail.rearrange("p (kx i j) -> p kx i j", kx=3, i=BPC)
        biv = band_int.rearrange("p (b m) -> p b m", b=9)
        for i in range(BPC):
            for c in range(C):
                p0 = i * 27 + c * 9
                src = biv[0:9, c * 3:c * 3 + 3, 0:TAIL]       # [9, 3kx, 8]
                nc.sync.dma_start(btv[p0:p0 + 9, :, i, :], src)

        # ---- shared tail tile (rows 504..511 of all 4 images) -------------
        xtail = const_pool.tile([108, CW], F16)
        nc.vector.memset(xtail[:, 0:1], 0.0)
        nc.vector.memset(xtail[:, CW - 1:CW], 0.0)
        r0 = NTILES * TILE_ROWS - 1  # 503
        for i in range(BPC):
            for c in range(C):
                p0 = i * 27 + c * 9
                nc.gpsimd.dma_start(xtail[p0:p0 + 9, 1:1 + W],
                                    xp[i, c, r0:r0 + 9, :])

        MT = BPC * TAIL  # 32 output rows
        ptail = psw_pool.tile([MT, W], F32, tag="ptail")
        for kx in range(3):
            nc.tensor.matmul(
                ptail[:],
                band_tail[0:108, kx * MT:(kx + 1) * MT],
                xtail[0:108, kx:kx + W],
                start=(kx == 0), stop=(kx == 2),
            )
        otail = out_pool.tile([MT, W], F16, tag="otail")
        nc.vector.tensor_scalar(out=otail[:], in0=ptail[:],
                                scalar1=thr[0:MT, 0:1], scalar2=None,
                                op0=mybir.AluOpType.is_ge)
        for i in range(BPC):
            nc.scalar.dma_start(yp[i, NTILES * TILE_ROWS:H, :],
                                otail[i * TAIL:(i + 1) * TAIL, :])

        # ---- main tiles, software-pipelined program order -----------------
        for i, (img, t) in enumerate(SETS):
            nxt = i + PRELOAD
            if nxt < len(SETS):
                emit_load(*SETS[nxt])
            emit_compute(img, t)

    nc.compile()
    return nc


def kernel(x: np.ndarray, w: np.ndarray, b: np.ndarray) -> np.ndarray:
    global LAST_EXEC_NS, LAST_RESULTS
    if "nc" not in _cache:
        _cache["nc"] = _build_nc()
    nc = _cache["nc"]

    x = np.ascontiguousarray(x, dtype=np.float32)
    w = np.ascontiguousarray(w, dtype=np.float32)
    b = np.ascontiguousarray(b, dtype=np.float32)
    in_maps = [
        {"x": x[i * BPC:(i + 1) * BPC], "w": w, "b": b} for i in range(NCORES)
    ]

    kwargs = {}
    if os.environ.get("BASS_CONV_TRACE", "") not in ("", "0"):
        try:
            import ntff_shim
            ntff_shim.install()
            kwargs["trace"] = True
        except Exception:
            pass

    res = None
    for attempt in range(3):
        try:
            res = run_bass_kernel_spmd(nc, in_maps,
                                       core_ids=list(range(NCORES)), **kwargs)
            break
        except Exception:
            if attempt == 2:
                raise
    LAST_EXEC_NS = res.exec_time_ns
    LAST_RESULTS = res
    out = np.concatenate([res.results[i]["out"][:, None, :, :]
                          for i in range(NCORES)], axis=0)
    return out.astype(np.float32)
